# revision 9
# baseline (speedup 1.0000x reference)
"""Trainium2 Bass kernel: dual-attention transformer block (nn_CustomBlock).

Reference semantics (per batch element b):
    q/k/v = x_b @ sa_w{q,k,v} + sa_b{q,k,v}
    sa    = softmax(q k^T / sqrt(DB)) v @ sa_wo + sa_bo
    x_b1  = x_b + sa
    q     = x_a @ ca_wq + ca_bq ; k/v = x_b1 @ ca_w{k,v} + ca_b{k,v}
    out   = x_b1 + softmax(q k^T / sqrt(DA)) v @ ca_wo + ca_bo

Sharding: data-parallel over batch — 8 batch elements, one per NeuronCore,
weights replicated.  No collectives.

All big matmuls run in fp8-e4m3 DoubleRow mode (2 contraction tiles per
instruction, 2x PE MAC rate vs bf16); PSUM accumulation is fp32, softmax is
fp32 on ACT, and the residual stream stays fp32.  Every fp8 operand is
pre-scaled into e4m3's comfortable normal range (host-measured maxima ~100
vs the 240 limit) and the inverse scales are folded into free spots:

  host:   w' = WS*w (all 8 weight mats), bq' = WS*bq, x fed as fp8 both
          layouts are host-transposed (xbT/xaT) so no on-device transpose.
  proj:   q_s/k_s/v_s = x@w' (+bq') in psum -> fp8     [= WS * q/k/v]
  scores: psum = q_s.k_s = WS^2 * qk; ACT Exp scale = sc/WS^2  (exact)
  softmax:wb = exp(..) bf16, row-sum fp32; one DVE tensor_scalar does
          wb*(1/sum)*QNS -> bf16, DMA-transpose, cast fp8     [= QNS*smax]
  AV:     psum = QNS*WS*attn; ACT Identity scale ATS -> fp8   [= 32*attn]
  outp:   psum = 32*WS*(attn@wo); ACT scale OPS -> bf16; DVE adds the
          fp32 residual.

Exact host-side bias folding (unchanged from the bf16 version):
  - k-bias shifts every score row by a constant -> softmax-invariant -> dropped.
  - v-bias passes through attention unchanged, so bv @ wo + bo folds into a
    per-feature vector added to the residual input (SA) / final output (CA).
  - q-bias applied on device via ACT bias in the q^T layout (host-scaled).

Softmax skips the max-subtraction: scaled scores stay in [-3, 3]; exp() in
fp32 is safe by a wide margin.  Host fp8 pipeline simulation on the real
inputs: rel fro err 1.1e-3 (gate 2e-2).
"""

import math
import os
from contextlib import ExitStack

import numpy as np
import ml_dtypes

import concourse.bass as bass
import concourse.mybir as mybir
import concourse.tile as tile
from concourse import bacc
from concourse.bass_utils import run_bass_kernel_spmd

P = 128
F32 = mybir.dt.float32
BF16 = mybir.dt.bfloat16
F8 = mybir.dt.float8e4
AF = mybir.ActivationFunctionType
ALU = mybir.AluOpType
DR = mybir.MatmulPerfMode.DoubleRow

B_FULL, N_FULL, DA_FULL, DB_FULL = 8, 2048, 768, 1024

WS = 32.0          # host weight scale: w' = WS*w
QNS = 4096.0       # softmax-weight fp8 scale (weights ~1/N would underflow e4m3)
ATS = 2.0 ** -12   # AV-psum (QNS*WS*attn) -> fp8 "32*attn"
OPS = 2.0 ** -10   # outproj-psum (32*WS*attn@wo) -> attn@wo


def build_block(tc, outs, ins, n, da, db):
    """Emit the dual-attention block into TileContext `tc`.

    ins/outs: dicts of DRAM APs:
      ins:  xbT [db,n] f8, xaT [da,n] f8 (host-transposed), xbpb [n,db] f32,
            sa_wq/sa_wk/sa_wv/sa_wo [db,db] f8, ca_wq [da,db] f8,
            ca_wk/ca_wv/ca_wo [db,db] f8 (all host-scaled by WS),
            bq_sa [P,db/P] f32, bq_ca [P,db/P] f32 (host-scaled by WS)
      outs: out [n,db] f32
    """
    nc = tc.nc
    KB, KA, NI = db // P, da // P, n // P
    MC = min(1024, n)         # projection m-chunk (columns of x^T); 2 psum banks
    NMC = n // MC
    PC = min(512, MC)         # one psum bank within a projection chunk
    NPC = MC // PC
    JH = min(1024, n)         # scores psum span (2 banks)
    NJH = n // JH
    JC = min(512, JH)         # one psum bank
    NJC = JH // JC
    SB = min(512, n)          # attention superblock (i columns per AV batch)
    NSB = n // SB
    IPSB = SB // P            # i-blocks per superblock
    EC = min(512, db)         # out-proj free chunk
    NEC = db // EC
    assert KB % 2 == 0 and KA % 2 == 0 and NI % 2 == 0, "DoubleRow needs even tiling"

    sc_sa = 1.0 / math.sqrt(float(db)) / (WS * WS)
    sc_ca = 1.0 / math.sqrt(float(da)) / (WS * WS)

    ctx = ExitStack()
    with ctx:
        sp = ctx.enter_context(tc.tile_pool(name="sp", bufs=1))
        pp = ctx.enter_context(tc.tile_pool(name="pp", bufs=1, space="PSUM"))
        dp = ctx.enter_context(tc.tile_pool(name="dp", bufs=1, space="DRAM"))

        # DRAM scratch
        xb1_d = dp.tile([n, db], F32, tag="xb1")
        xb1b_d = dp.tile([n, db], BF16, tag="xb1b")

        # persistent SBUF
        kT = sp.tile([P, KB, n], F8, tag="kT")          # k^T  [feat, seq]
        qt_sa = sp.tile([P, KB, n], F8, tag="qt_sa")    # q^T  [feat, seq]
        qt_ca = sp.tile([P, KB, n], F8, tag="qt_ca")
        v_sb = sp.tile([P, NI, db], F8, tag="v")        # v    [seq, feat]
        bqs = sp.tile([P, KB], F32, tag="bqs")
        bqc = sp.tile([P, KB], F32, tag="bqc")
        zb = sp.tile([P, 1], F32, tag="zb")
        nc.sync.dma_start(bqs[:], ins["bq_sa"][:])
        nc.sync.dma_start(bqc[:], ins["bq_ca"][:])
        nc.gpsimd.memset(zb[:], 0.0)

        def load_w(name, ktiles):
            # two half-loads (pair-aligned): consumers of the first k-pairs
            # start after half the matrix is in (Tile tracks subtile writes)
            # bufs=3: wv/wq/wk are all live during the fused SA projection pass
            wt = sp.tile([P, ktiles, db], F8, tag="w", bufs=3)
            src = ins[name].rearrange("(t p) e -> p t e", p=P)
            h = min(ktiles, 2 * ((ktiles + 3) // 4) or 2)
            nc.sync.dma_start(wt[:, :h, :], src[:, :h, :])
            if h < ktiles:
                nc.sync.dma_start(wt[:, h:, :], src[:, h:, :])
            return wt

        def load_xT(srcT, ktiles, mcc):
            # host-transposed fp8 x^T chunk [p, kt, m] with k = kt*P + p
            xT = sp.tile([P, ktiles, MC], F8, tag="xcwt", bufs=2)
            nc.sync.dma_start(
                xT[:],
                srcT.rearrange("(t p) m -> p t m", p=P)[:, :, mcc * MC:(mcc + 1) * MC],
            )
            return xT

        def xpose_cast_chunk(src_bf, ktiles, mcc):
            # device-produced x (bf16 in DRAM) -> transposed fp8 chunk: the
            # DMA XBAR only transposes 2-byte elements, so bf16 then DVE-cast
            xTb = sp.tile([P, ktiles, MC], BF16, tag="xtb", bufs=2)
            nc.sync.dma_start_transpose(xTb[:], src_bf[mcc * MC:(mcc + 1) * MC, :])
            xT = sp.tile([P, ktiles, MC], F8, tag="xcwt", bufs=2)
            nc.vector.tensor_copy(xT[:], xTb[:])
            return xT

        def proj_v_chunk(w_sb, xT, ktiles, mcc):
            # v[m, e] = sum_k x[m, k] w[k, e]  (natural layout, into v_sb).
            # One [P, db] psum spans all e-chunks: each stationary load (the
            # x-slice pair) serves NEC matmuls instead of one.
            for q2 in range(MC // P):
                mt = mcc * (MC // P) + q2
                ps = pp.tile([P, db], F32, tag="ps_s", bufs=2)
                for kp in range(ktiles // 2):
                    for ecc in range(NEC):
                        nc.tensor.matmul(
                            ps[:, ecc * EC:(ecc + 1) * EC],
                            xT[:, 2 * kp:2 * kp + 2, q2 * P:(q2 + 1) * P],
                            w_sb[:, 2 * kp:2 * kp + 2, ecc * EC:(ecc + 1) * EC],
                            start=(kp == 0), stop=(kp == ktiles // 2 - 1),
                            perf_mode=DR,
                        )
                nc.vector.tensor_copy(v_sb[:, mt, :], ps[:])

        def proj_T_block(w_sb, ktiles, xT, nt, mcc, sink):
            # out^T[f, m] = sum_k w[k, f] x^T[k, m] for f-tile nt, m-chunk mcc.
            # One [P, MC] psum spans NPC m-halves: each stationary load (the
            # w-slice pair) serves NPC matmuls instead of one.
            ps = pp.tile([P, MC], F32, tag="ps_s", bufs=2)
            for kp in range(ktiles // 2):
                for jc in range(NPC):
                    nc.tensor.matmul(
                        ps[:, jc * PC:(jc + 1) * PC],
                        w_sb[:, 2 * kp:2 * kp + 2, nt * P:(nt + 1) * P],
                        xT[:, 2 * kp:2 * kp + 2, jc * PC:(jc + 1) * PC],
                        start=(kp == 0), stop=(kp == ktiles // 2 - 1),
                        perf_mode=DR,
                    )
            sink(nt, mcc, ps)

        def q_sink(qt_sb, bq_tile):
            def sink(nt, mcc, ps):
                # q^T stays SBUF-resident (16KB/partition in fp8): the scores
                # phase slices it directly — no DRAM round trip
                nc.scalar.activation(qt_sb[:, nt, mcc * MC:(mcc + 1) * MC], ps[:],
                                     AF.Identity, bias=bq_tile[:, nt:nt + 1])
            return sink

        def k_sink(nt, mcc, ps):
            nc.vector.tensor_copy(kT[:, nt, mcc * MC:(mcc + 1) * MC], ps[:])

        def attention(qt_sb, scale, wo_sb, resid_dram, writer):
            # Software-pipelined over superblocks: the scores/softmax/transpose
            # chain of superblock sbi is emitted BEFORE the AV/out-proj of
            # sbi-1, so the PE never stalls on the (ACT/DVE/DMA) softmax tail.
            def scores_phase(sbi, wt_t):
                for q3 in range(IPSB):
                    ib = sbi * IPSB + q3
                    wb_t = sp.tile([P, n], BF16, tag="wb", bufs=2)
                    ss_t = sp.tile([P, NJH], F32, tag="ss", bufs=2)
                    for jh in range(NJH):
                        ps_s = pp.tile([P, JH], F32, tag="ps_s", bufs=2)
                        for kp in range(KB // 2):
                            for jc in range(NJC):
                                nc.tensor.matmul(
                                    ps_s[:, jc * JC:(jc + 1) * JC],
                                    qt_sb[:, 2 * kp:2 * kp + 2, ib * P:(ib + 1) * P],
                                    kT[:, 2 * kp:2 * kp + 2,
                                       jh * JH + jc * JC:jh * JH + (jc + 1) * JC],
                                    start=(kp == 0), stop=(kp == KB // 2 - 1),
                                    perf_mode=DR,
                                )
                        nc.scalar.activation(
                            wb_t[:, jh * JH:(jh + 1) * JH], ps_s[:], AF.Exp,
                            bias=zb[:], scale=scale,
                            accum_out=ss_t[:, jh:jh + 1],
                        )
                    rr = sp.tile([P, 1], F32, tag="rr", bufs=2)
                    if NJH > 1:
                        rs = sp.tile([P, 1], F32, tag="rs", bufs=2)
                        nc.vector.tensor_reduce(rs[:], ss_t[:], axis=mybir.AxisListType.X, op=ALU.add)
                        nc.vector.reciprocal(rr[:], rs[:])
                    else:
                        nc.vector.reciprocal(rr[:], ss_t[:])
                    # normalize + lift into fp8 range: wb = wb * (1/sum) * QNS
                    nc.vector.tensor_scalar(wb_t[:], wb_t[:], rr[:, 0:1], QNS, ALU.mult, ALU.mult)
                    # transpose the scaled weights: w[i, j] -> wT[j, i]
                    wtb = sp.tile([P, NI, P], BF16, tag="wtb", bufs=2)
                    nc.sync.dma_start_transpose(wtb[:], wb_t[:])
                    nc.vector.tensor_copy(wt_t[:, :, q3 * P:(q3 + 1) * P], wtb[:])

            def av_part(sbi, wt_t):
                # attn^T[d, i] = sum_j v[j, d] wT[j, i]
                at_t = sp.tile([P, KB, SB], F8, tag="at", bufs=2)
                for dt in range(KB):
                    ps_a = pp.tile([P, SB], F32, tag="ps_a", bufs=2)
                    for jp in range(NI // 2):
                        nc.tensor.matmul(
                            ps_a[:],
                            v_sb[:, 2 * jp:2 * jp + 2, dt * P:(dt + 1) * P],
                            wt_t[:, 2 * jp:2 * jp + 2, :],
                            start=(jp == 0), stop=(jp == NI // 2 - 1),
                            perf_mode=DR,
                        )
                    nc.scalar.activation(at_t[:, dt, :], ps_a[:], AF.Identity,
                                         bias=zb[:], scale=ATS)
                return at_t

            def op_part(sbi, at_t):
                # out-proj (rescaled on ACT) + fp32 residual add on DVE
                for q3 in range(IPSB):
                    ib = sbi * IPSB + q3
                    rx = sp.tile([P, db], F32, tag="rx", bufs=2)
                    nc.sync.dma_start(rx[:], resid_dram[ib * P:(ib + 1) * P, :])
                    ro = sp.tile([P, db], F32, tag="ro", bufs=2)
                    po = sp.tile([P, db], BF16, tag="po", bufs=2)
                    for ecc in range(NEC):
                        ps_o = pp.tile([P, EC], F32, tag="pj", bufs=2)
                        for dp2 in range(KB // 2):
                            nc.tensor.matmul(
                                ps_o[:],
                                at_t[:, 2 * dp2:2 * dp2 + 2, q3 * P:(q3 + 1) * P],
                                wo_sb[:, 2 * dp2:2 * dp2 + 2, ecc * EC:(ecc + 1) * EC],
                                start=(dp2 == 0), stop=(dp2 == KB // 2 - 1),
                                perf_mode=DR,
                            )
                        nc.scalar.activation(po[:, ecc * EC:(ecc + 1) * EC], ps_o[:],
                                             AF.Identity, bias=zb[:], scale=OPS)
                        nc.vector.tensor_tensor(
                            ro[:, ecc * EC:(ecc + 1) * EC], po[:, ecc * EC:(ecc + 1) * EC],
                            rx[:, ecc * EC:(ecc + 1) * EC], ALU.add,
                        )
                    writer(ib, ro)

            pend_av = None   # (sbi, wt_t) awaiting AV
            pend_op = None   # (sbi, at_t) awaiting out-proj
            for sbi in range(NSB):
                wt_t = sp.tile([P, NI, SB], F8, tag="xcwt", bufs=2)
                scores_phase(sbi, wt_t)
                new_at = av_part(*pend_av) if pend_av is not None else None
                if pend_op is not None:
                    op_part(*pend_op)
                if new_at is not None:
                    pend_op = (pend_av[0], new_at)
                pend_av = (sbi, wt_t)
            at_t = av_part(*pend_av)
            if pend_op is not None:
                op_part(*pend_op)
            op_part(pend_av[0], at_t)

        def sa_writer(ib, ro):
            nc.sync.dma_start(xb1_d[ib * P:(ib + 1) * P, :], ro[:])
            rb = sp.tile([P, db], BF16, tag="rb", bufs=2)
            nc.scalar.activation(rb[:], ro[:], AF.Copy)
            nc.sync.dma_start(xb1b_d[ib * P:(ib + 1) * P, :], rb[:])

        def ca_writer(ib, ro):
            nc.sync.dma_start(outs["out"][ib * P:(ib + 1) * P, :], ro[:])

        # CA-q depends only on x_a — emit it first: smallest startup loads,
        # and it decouples the SA->CA boundary entirely.
        wq2 = load_w("ca_wq", KA)
        sink_q_ca = q_sink(qt_ca, bqc)
        for mcc in range(NMC):
            xTa = load_xT(ins["xaT"], KA, mcc)
            for nt in range(KB):
                proj_T_block(wq2, KA, xTa, nt, mcc, sink_q_ca)

        # ===================== self-attention =====================
        # v/q/k share each xbT chunk load (one DMA pass over x_b)
        wv = load_w("sa_wv", KB)
        wq = load_w("sa_wq", KB)
        wk = load_w("sa_wk", KB)
        sink_q_sa = q_sink(qt_sa, bqs)
        for mcc in range(NMC):
            xT = load_xT(ins["xbT"], KB, mcc)
            proj_v_chunk(wv, xT, KB, mcc)
            for nt in range(KB):
                proj_T_block(wq, KB, xT, nt, mcc, sink_q_sa)
                proj_T_block(wk, KB, xT, nt, mcc, k_sink)

        wo = load_w("sa_wo", KB)
        attention(qt_sa, sc_sa, wo, ins["xbpb"], sa_writer)

        # ===================== cross-attention =====================
        # v and k share each transposed xb1 chunk (one transpose instead of
        # two, and 2x the PE work per chunk keeps the chunk ring ahead).
        wv2 = load_w("ca_wv", KB)
        wk2 = load_w("ca_wk", KB)
        for mcc in range(NMC):
            xTb = xpose_cast_chunk(xb1b_d, KB, mcc)
            proj_v_chunk(wv2, xTb, KB, mcc)
            for nt in range(KB):
                proj_T_block(wk2, KB, xTb, nt, mcc, k_sink)
        wo2 = load_w("ca_wo", KB)
        attention(qt_ca, sc_ca, wo2, xb1_d, ca_writer)


def build_program(n=N_FULL, da=DA_FULL, db=DB_FULL, repeat=1):
    """Build the single-core Bass program; returns the Bass module.

    repeat>1 re-emits the whole block body N times (idempotent — same inputs
    and scratch): used to measure per-iteration device time above the fixed
    dispatch overhead."""
    nc = bacc.Bacc("TRN2", target_bir_lowering=False, debug=False, enable_asserts=False)
    KB = db // P
    ins = {
        "xbT": nc.dram_tensor("xbT", [db, n], F8, kind="ExternalInput").ap(),
        "xaT": nc.dram_tensor("xaT", [da, n], F8, kind="ExternalInput").ap(),
        "xbpb": nc.dram_tensor("xbpb", [n, db], F32, kind="ExternalInput").ap(),
        "sa_wq": nc.dram_tensor("sa_wq", [db, db], F8, kind="ExternalInput").ap(),
        "sa_wk": nc.dram_tensor("sa_wk", [db, db], F8, kind="ExternalInput").ap(),
        "sa_wv": nc.dram_tensor("sa_wv", [db, db], F8, kind="ExternalInput").ap(),
        "sa_wo": nc.dram_tensor("sa_wo", [db, db], F8, kind="ExternalInput").ap(),
        "ca_wq": nc.dram_tensor("ca_wq", [da, db], F8, kind="ExternalInput").ap(),
        "ca_wk": nc.dram_tensor("ca_wk", [db, db], F8, kind="ExternalInput").ap(),
        "ca_wv": nc.dram_tensor("ca_wv", [db, db], F8, kind="ExternalInput").ap(),
        "ca_wo": nc.dram_tensor("ca_wo", [db, db], F8, kind="ExternalInput").ap(),
        "bq_sa": nc.dram_tensor("bq_sa", [P, KB], F32, kind="ExternalInput").ap(),
        "bq_ca": nc.dram_tensor("bq_ca", [P, KB], F32, kind="ExternalInput").ap(),
    }
    outs = {"out": nc.dram_tensor("out", [n, db], F32, kind="ExternalOutput").ap()}
    with tile.TileContext(nc) as tc:
        if repeat == 1:
            build_block(tc, outs, ins, n, da, db)
        else:
            # hardware loop: same NEFF size at any repeat; an all-engine
            # barrier separates iterations (body is idempotent)
            with tc.For_i(0, repeat):
                build_block(tc, outs, ins, n, da, db)
    nc.compile()
    return nc


def prepare_maps(inputs, n=N_FULL, da=DA_FULL, db=DB_FULL):
    """Host-side prep: fp8 casts/scaling, transposes, exact bias folding.

    Returns (in_maps, add_out)."""
    f8 = ml_dtypes.float8_e4m3
    f32 = np.float32
    g = {k: np.ascontiguousarray(np.asarray(v)) for k, v in inputs.items()}
    nb = g["x_a"].shape[0]

    # exact folds (see module docstring); all biases are added in fp32
    b_eff_sa = (g["sa_bv"].astype(f32) @ g["sa_wo"].astype(f32) + g["sa_bo"].astype(f32))
    b_eff_ca = (g["ca_bv"].astype(f32) @ g["ca_wo"].astype(f32) + g["ca_bo"].astype(f32))
    xbpb = (g["x_b"].astype(f32) + b_eff_sa[None, None, :]).astype(f32)

    KB = db // P
    ws = f32(WS)
    common = {
        name: np.ascontiguousarray((ws * g[name].astype(f32)).astype(f8))
        for name in ("sa_wq", "sa_wk", "sa_wv", "sa_wo",
                     "ca_wq", "ca_wk", "ca_wv", "ca_wo")
    }
    common["bq_sa"] = np.ascontiguousarray((ws * g["sa_bq"].astype(f32)).reshape(KB, P).T)
    common["bq_ca"] = np.ascontiguousarray((ws * g["ca_bq"].astype(f32)).reshape(KB, P).T)
    in_maps = []
    for b in range(nb):
        in_maps.append(dict(
            xbT=np.ascontiguousarray(g["x_b"][b].astype(f32).T.astype(f8)),
            xaT=np.ascontiguousarray(g["x_a"][b].astype(f32).T.astype(f8)),
            xbpb=np.ascontiguousarray(xbpb[b]),
            **common,
        ))
    return in_maps, b_eff_ca


_CACHE = {}


def run_on_device(inputs, trace=False, **run_kwargs):
    """Run the full problem on 8 NeuronCores.  Returns (out [B,N,DB] f32, results)."""
    if not trace:
        # NTFF tracing needs antenv.axon_hooks, absent in this container; make
        # sure an inherited BASS_TRACE=1 can't route us into that path.
        os.environ.setdefault("BASS_NEVER_TRACE", "1")
    if "nc" not in _CACHE:
        _CACHE["nc"] = build_program()
    nc = _CACHE["nc"]
    in_maps, add_out = prepare_maps(inputs)
    res = run_bass_kernel_spmd(
        nc, in_maps, core_ids=list(range(len(in_maps))), trace=trace, **run_kwargs,
    )
    out = np.stack([r["out"] for r in res.results], axis=0)
    out = (out + add_out[None, None, :]).astype(np.float32)
    return out, res


def kernel(**inputs) -> np.ndarray:
    out, _ = run_on_device(inputs)
    return out


# revision 10
# speedup vs baseline: 1.0492x; 1.0492x over previous
"""Trainium2 Bass kernel: dual-attention transformer block (nn_CustomBlock).

Reference semantics (per batch element b):
    q/k/v = x_b @ sa_w{q,k,v} + sa_b{q,k,v}
    sa    = softmax(q k^T / sqrt(DB)) v @ sa_wo + sa_bo
    x_b1  = x_b + sa
    q     = x_a @ ca_wq + ca_bq ; k/v = x_b1 @ ca_w{k,v} + ca_b{k,v}
    out   = x_b1 + softmax(q k^T / sqrt(DA)) v @ ca_wo + ca_bo

Sharding: data-parallel over batch — 8 batch elements, one per NeuronCore,
weights replicated.  No collectives.

All big matmuls run in fp8-e4m3 DoubleRow mode (2 contraction tiles per
instruction, 2x PE MAC rate vs bf16); PSUM accumulation is fp32, softmax is
fp32 on ACT, and the residual stream stays fp32.  Every fp8 operand is
pre-scaled into e4m3's comfortable normal range (host-measured maxima ~100
vs the 240 limit) and the inverse scales are folded into free spots:

  host:   w' = WS*w (all 8 weight mats), bq' = WS*bq, x fed as fp8 both
          layouts are host-transposed (xbT/xaT) so no on-device transpose.
  proj:   q_s/k_s/v_s = x@w' (+bq') in psum -> fp8     [= WS * q/k/v]
  scores: psum = q_s.k_s = WS^2 * qk; ACT Exp scale = sc/WS^2  (exact)
  softmax:wb = exp(..) bf16, row-sum fp32; one DVE tensor_scalar does
          wb*(1/sum)*QNS -> bf16, DMA-transpose, cast fp8     [= QNS*smax]
  AV:     psum = QNS*WS*attn; ACT Identity scale ATS -> fp8   [= 32*attn]
  outp:   psum = 32*WS*(attn@wo); ACT scale OPS -> bf16; DVE adds the
          fp32 residual.

Exact host-side bias folding (unchanged from the bf16 version):
  - k-bias shifts every score row by a constant -> softmax-invariant -> dropped.
  - v-bias passes through attention unchanged, so bv @ wo + bo folds into a
    per-feature vector added to the residual input (SA) / final output (CA).
  - q-bias applied on device via ACT bias in the q^T layout (host-scaled).

Softmax skips the max-subtraction: scaled scores stay in [-3, 3]; exp() in
fp32 is safe by a wide margin.  Host fp8 pipeline simulation on the real
inputs: rel fro err 1.1e-3 (gate 2e-2).
"""

import math
import os
from contextlib import ExitStack

import numpy as np
import ml_dtypes

import concourse.bass as bass
import concourse.mybir as mybir
import concourse.tile as tile
from concourse import bacc
from concourse.bass_utils import run_bass_kernel_spmd

P = 128
F32 = mybir.dt.float32
BF16 = mybir.dt.bfloat16
F8 = mybir.dt.float8e4
AF = mybir.ActivationFunctionType
ALU = mybir.AluOpType
DR = mybir.MatmulPerfMode.DoubleRow

B_FULL, N_FULL, DA_FULL, DB_FULL = 8, 2048, 768, 1024

WS = 32.0          # host weight scale: w' = WS*w
QNS = 4096.0       # softmax-weight fp8 scale (weights ~1/N would underflow e4m3)
ATS = 2.0 ** -12   # AV-psum (QNS*WS*attn) -> fp8 "32*attn"
OPS = 2.0 ** -10   # outproj-psum (32*WS*attn@wo) -> attn@wo


def build_block(tc, outs, ins, n, da, db):
    """Emit the dual-attention block into TileContext `tc`.

    ins/outs: dicts of DRAM APs:
      ins:  xbT [db,n] f8, xaT [da,n] f8 (host-transposed), xbpb [n,db] f32,
            sa_wq/sa_wk/sa_wv/sa_wo [db,db] f8, ca_wq [da,db] f8,
            ca_wk/ca_wv/ca_wo [db,db] f8 (all host-scaled by WS),
            bq_sa [P,db/P] f32, bq_ca [P,db/P] f32 (host-scaled by WS)
      outs: out [n,db] f32
    """
    nc = tc.nc
    KB, KA, NI = db // P, da // P, n // P
    MC = min(1024, n)         # projection m-chunk (columns of x^T); 2 psum banks
    NMC = n // MC
    PC = min(512, MC)         # one psum bank within a projection chunk
    NPC = MC // PC
    JH = min(1024, n)         # scores psum span (2 banks)
    NJH = n // JH
    JC = min(512, JH)         # one psum bank
    NJC = JH // JC
    SB = min(512, n)          # attention superblock (i columns per AV batch)
    NSB = n // SB
    IPSB = SB // P            # i-blocks per superblock
    EC = min(512, db)         # out-proj free chunk
    NEC = db // EC
    assert KB % 2 == 0 and KA % 2 == 0 and NI % 2 == 0, "DoubleRow needs even tiling"

    sc_sa = 1.0 / math.sqrt(float(db)) / (WS * WS)
    sc_ca = 1.0 / math.sqrt(float(da)) / (WS * WS)

    ctx = ExitStack()
    with ctx:
        sp = ctx.enter_context(tc.tile_pool(name="sp", bufs=1))
        pp = ctx.enter_context(tc.tile_pool(name="pp", bufs=1, space="PSUM"))
        dp = ctx.enter_context(tc.tile_pool(name="dp", bufs=1, space="DRAM"))

        # DRAM scratch
        xb1_d = dp.tile([n, db], F32, tag="xb1")
        xb1b_d = dp.tile([n, db], BF16, tag="xb1b")

        # persistent SBUF
        kT = sp.tile([P, KB, n], F8, tag="kT")          # k^T  [feat, seq]
        qt_sa = sp.tile([P, KB, n], F8, tag="qt_sa")    # q^T  [feat, seq]
        qt_ca = sp.tile([P, KB, n], F8, tag="qt_ca")
        v_sb = sp.tile([P, NI, db], F8, tag="v")        # v    [seq, feat]
        bqs = sp.tile([P, KB], F32, tag="bqs")
        bqc = sp.tile([P, KB], F32, tag="bqc")
        zb = sp.tile([P, 1], F32, tag="zb")
        nc.sync.dma_start(bqs[:], ins["bq_sa"][:])
        nc.sync.dma_start(bqc[:], ins["bq_ca"][:])
        nc.gpsimd.memset(zb[:], 0.0)

        def load_w(name, ktiles):
            # two half-loads (pair-aligned): consumers of the first k-pairs
            # start after half the matrix is in (Tile tracks subtile writes)
            # bufs=3: wv/wq/wk are all live during the fused SA projection pass
            wt = sp.tile([P, ktiles, db], F8, tag="w", bufs=3)
            src = ins[name].rearrange("(t p) e -> p t e", p=P)
            h = min(ktiles, 2 * ((ktiles + 3) // 4) or 2)
            nc.sync.dma_start(wt[:, :h, :], src[:, :h, :])
            if h < ktiles:
                nc.sync.dma_start(wt[:, h:, :], src[:, h:, :])
            return wt

        def load_xT(srcT, ktiles, mcc):
            # host-transposed fp8 x^T chunk [p, kt, m] with k = kt*P + p
            xT = sp.tile([P, ktiles, MC], F8, tag="xcwt", bufs=2)
            nc.sync.dma_start(
                xT[:],
                srcT.rearrange("(t p) m -> p t m", p=P)[:, :, mcc * MC:(mcc + 1) * MC],
            )
            return xT

        def xpose_cast_chunk(src_bf, ktiles, mcc):
            # device-produced x (bf16 in DRAM) -> transposed fp8 chunk: the
            # DMA XBAR only transposes 2-byte elements, so bf16 then DVE-cast
            xTb = sp.tile([P, ktiles, MC], BF16, tag="xtb", bufs=2)
            nc.sync.dma_start_transpose(xTb[:], src_bf[mcc * MC:(mcc + 1) * MC, :])
            xT = sp.tile([P, ktiles, MC], F8, tag="xcwt", bufs=2)
            nc.vector.tensor_copy(xT[:], xTb[:])
            return xT

        def proj_v_chunk(w_sb, xT, ktiles, mcc):
            # v[m, e] = sum_k x[m, k] w[k, e]  (natural layout, into v_sb).
            # One [P, db] psum spans all e-chunks: each stationary load (the
            # x-slice pair) serves NEC matmuls instead of one.
            for q2 in range(MC // P):
                mt = mcc * (MC // P) + q2
                ps = pp.tile([P, db], F32, tag="ps_s", bufs=2)
                for kp in range(ktiles // 2):
                    for ecc in range(NEC):
                        nc.tensor.matmul(
                            ps[:, ecc * EC:(ecc + 1) * EC],
                            xT[:, 2 * kp:2 * kp + 2, q2 * P:(q2 + 1) * P],
                            w_sb[:, 2 * kp:2 * kp + 2, ecc * EC:(ecc + 1) * EC],
                            start=(kp == 0), stop=(kp == ktiles // 2 - 1),
                            perf_mode=DR,
                        )
                nc.vector.tensor_copy(v_sb[:, mt, :], ps[:])

        def proj_T_block(w_sb, ktiles, xT, nt, mcc, sink):
            # out^T[f, m] = sum_k w[k, f] x^T[k, m] for f-tile nt, m-chunk mcc.
            # One [P, MC] psum spans NPC m-halves: each stationary load (the
            # w-slice pair) serves NPC matmuls instead of one.
            ps = pp.tile([P, MC], F32, tag="ps_s", bufs=2)
            for kp in range(ktiles // 2):
                for jc in range(NPC):
                    nc.tensor.matmul(
                        ps[:, jc * PC:(jc + 1) * PC],
                        w_sb[:, 2 * kp:2 * kp + 2, nt * P:(nt + 1) * P],
                        xT[:, 2 * kp:2 * kp + 2, jc * PC:(jc + 1) * PC],
                        start=(kp == 0), stop=(kp == ktiles // 2 - 1),
                        perf_mode=DR,
                    )
            sink(nt, mcc, ps)

        def q_sink(qt_sb, bq_tile):
            def sink(nt, mcc, ps):
                # q^T stays SBUF-resident (16KB/partition in fp8): the scores
                # phase slices it directly — no DRAM round trip
                nc.scalar.activation(qt_sb[:, nt, mcc * MC:(mcc + 1) * MC], ps[:],
                                     AF.Identity, bias=bq_tile[:, nt:nt + 1])
            return sink

        def k_sink(nt, mcc, ps):
            nc.vector.tensor_copy(kT[:, nt, mcc * MC:(mcc + 1) * MC], ps[:])

        def attention(qt_sb, scale, wo_sb, resid_dram, writer):
            # Software-pipelined over superblocks: the scores/softmax/transpose
            # chain of superblock sbi is emitted BEFORE the AV/out-proj of
            # sbi-1, so the PE never stalls on the (ACT/DVE/DMA) softmax tail.
            def scores_phase(sbi, wt_t):
                for q3 in range(IPSB):
                    ib = sbi * IPSB + q3
                    wb_t = sp.tile([P, n], BF16, tag="wb", bufs=2)
                    ss_t = sp.tile([P, NJH], F32, tag="ss", bufs=2)
                    for jh in range(NJH):
                        ps_s = pp.tile([P, JH], F32, tag="ps_s", bufs=2)
                        for kp in range(KB // 2):
                            for jc in range(NJC):
                                nc.tensor.matmul(
                                    ps_s[:, jc * JC:(jc + 1) * JC],
                                    qt_sb[:, 2 * kp:2 * kp + 2, ib * P:(ib + 1) * P],
                                    kT[:, 2 * kp:2 * kp + 2,
                                       jh * JH + jc * JC:jh * JH + (jc + 1) * JC],
                                    start=(kp == 0), stop=(kp == KB // 2 - 1),
                                    perf_mode=DR,
                                )
                        nc.scalar.activation(
                            wb_t[:, jh * JH:(jh + 1) * JH], ps_s[:], AF.Exp,
                            bias=zb[:], scale=scale,
                            accum_out=ss_t[:, jh:jh + 1],
                        )
                    rr = sp.tile([P, 1], F32, tag="rr", bufs=2)
                    if NJH > 1:
                        rs = sp.tile([P, 1], F32, tag="rs", bufs=2)
                        nc.vector.tensor_reduce(rs[:], ss_t[:], axis=mybir.AxisListType.X, op=ALU.add)
                        nc.vector.reciprocal(rr[:], rs[:])
                    else:
                        nc.vector.reciprocal(rr[:], ss_t[:])
                    # normalize + lift into fp8 range: wb = wb * (1/sum) * QNS
                    nc.vector.tensor_scalar(wb_t[:], wb_t[:], rr[:, 0:1], QNS, ALU.mult, ALU.mult)
                    # transpose the scaled weights: w[i, j] -> wT[j, i]
                    wtb = sp.tile([P, NI, P], BF16, tag="wtb", bufs=2)
                    nc.sync.dma_start_transpose(wtb[:], wb_t[:])
                    nc.vector.tensor_copy(wt_t[:, :, q3 * P:(q3 + 1) * P], wtb[:])

            def av_part(sbi, wt_t):
                # attn^T[d, i] = sum_j v[j, d] wT[j, i]
                at_t = sp.tile([P, KB, SB], F8, tag="at", bufs=2)
                for dt in range(KB):
                    ps_a = pp.tile([P, SB], F32, tag="ps_a", bufs=2)
                    for jp in range(NI // 2):
                        nc.tensor.matmul(
                            ps_a[:],
                            v_sb[:, 2 * jp:2 * jp + 2, dt * P:(dt + 1) * P],
                            wt_t[:, 2 * jp:2 * jp + 2, :],
                            start=(jp == 0), stop=(jp == NI // 2 - 1),
                            perf_mode=DR,
                        )
                    nc.scalar.activation(at_t[:, dt, :], ps_a[:], AF.Identity,
                                         bias=zb[:], scale=ATS)
                return at_t

            def op_part(sbi, at_t):
                # out-proj (rescaled on ACT) + fp32 residual add on DVE
                for q3 in range(IPSB):
                    ib = sbi * IPSB + q3
                    rx = sp.tile([P, db], F32, tag="rx", bufs=2)
                    nc.sync.dma_start(rx[:], resid_dram[ib * P:(ib + 1) * P, :])
                    ro = sp.tile([P, db], F32, tag="ro", bufs=2)
                    po = sp.tile([P, db], BF16, tag="po", bufs=2)
                    for ecc in range(NEC):
                        ps_o = pp.tile([P, EC], F32, tag="pj", bufs=2)
                        for dp2 in range(KB // 2):
                            nc.tensor.matmul(
                                ps_o[:],
                                at_t[:, 2 * dp2:2 * dp2 + 2, q3 * P:(q3 + 1) * P],
                                wo_sb[:, 2 * dp2:2 * dp2 + 2, ecc * EC:(ecc + 1) * EC],
                                start=(dp2 == 0), stop=(dp2 == KB // 2 - 1),
                                perf_mode=DR,
                            )
                        nc.scalar.activation(po[:, ecc * EC:(ecc + 1) * EC], ps_o[:],
                                             AF.Identity, bias=zb[:], scale=OPS)
                        nc.vector.tensor_tensor(
                            ro[:, ecc * EC:(ecc + 1) * EC], po[:, ecc * EC:(ecc + 1) * EC],
                            rx[:, ecc * EC:(ecc + 1) * EC], ALU.add,
                        )
                    writer(ib, ro)

            pend_av = None   # (sbi, wt_t) awaiting AV
            pend_op = None   # (sbi, at_t) awaiting out-proj
            for sbi in range(NSB):
                wt_t = sp.tile([P, NI, SB], F8, tag="xcwt", bufs=2)
                scores_phase(sbi, wt_t)
                new_at = av_part(*pend_av) if pend_av is not None else None
                if pend_op is not None:
                    op_part(*pend_op)
                if new_at is not None:
                    pend_op = (pend_av[0], new_at)
                pend_av = (sbi, wt_t)
            at_t = av_part(*pend_av)
            if pend_op is not None:
                op_part(*pend_op)
            op_part(pend_av[0], at_t)

        def sa_writer(ib, ro):
            nc.sync.dma_start(xb1_d[ib * P:(ib + 1) * P, :], ro[:])
            rb = sp.tile([P, db], BF16, tag="rb", bufs=2)
            nc.scalar.activation(rb[:], ro[:], AF.Copy)
            nc.sync.dma_start(xb1b_d[ib * P:(ib + 1) * P, :], rb[:])

        def ca_writer(ib, ro):
            nc.sync.dma_start(outs["out"][ib * P:(ib + 1) * P, :], ro[:])

        # CA-q depends only on x_a — emit it first: smallest startup loads,
        # and it decouples the SA->CA boundary entirely.
        wq2 = load_w("ca_wq", KA)
        sink_q_ca = q_sink(qt_ca, bqc)
        for mcc in range(NMC):
            xTa = load_xT(ins["xaT"], KA, mcc)
            for nt in range(KB):
                proj_T_block(wq2, KA, xTa, nt, mcc, sink_q_ca)

        # ===================== self-attention =====================
        wv = load_w("sa_wv", KB)
        for mcc in range(NMC):
            proj_v_chunk(wv, load_xT(ins["xbT"], KB, mcc), KB, mcc)
        wq = load_w("sa_wq", KB)
        wk = load_w("sa_wk", KB)
        sink_q_sa = q_sink(qt_sa, bqs)
        for mcc in range(NMC):
            xT = load_xT(ins["xbT"], KB, mcc)
            for nt in range(KB):
                proj_T_block(wq, KB, xT, nt, mcc, sink_q_sa)
                proj_T_block(wk, KB, xT, nt, mcc, k_sink)

        wo = load_w("sa_wo", KB)
        attention(qt_sa, sc_sa, wo, ins["xbpb"], sa_writer)

        # ===================== cross-attention =====================
        # v and k share each transposed xb1 chunk (one transpose instead of
        # two, and 2x the PE work per chunk keeps the chunk ring ahead).
        wv2 = load_w("ca_wv", KB)
        wk2 = load_w("ca_wk", KB)
        for mcc in range(NMC):
            xTb = xpose_cast_chunk(xb1b_d, KB, mcc)
            proj_v_chunk(wv2, xTb, KB, mcc)
            for nt in range(KB):
                proj_T_block(wk2, KB, xTb, nt, mcc, k_sink)
        wo2 = load_w("ca_wo", KB)
        attention(qt_ca, sc_ca, wo2, xb1_d, ca_writer)


def build_program(n=N_FULL, da=DA_FULL, db=DB_FULL, repeat=1):
    """Build the single-core Bass program; returns the Bass module.

    repeat>1 re-emits the whole block body N times (idempotent — same inputs
    and scratch): used to measure per-iteration device time above the fixed
    dispatch overhead."""
    nc = bacc.Bacc("TRN2", target_bir_lowering=False, debug=False, enable_asserts=False)
    KB = db // P
    ins = {
        "xbT": nc.dram_tensor("xbT", [db, n], F8, kind="ExternalInput").ap(),
        "xaT": nc.dram_tensor("xaT", [da, n], F8, kind="ExternalInput").ap(),
        "xbpb": nc.dram_tensor("xbpb", [n, db], F32, kind="ExternalInput").ap(),
        "sa_wq": nc.dram_tensor("sa_wq", [db, db], F8, kind="ExternalInput").ap(),
        "sa_wk": nc.dram_tensor("sa_wk", [db, db], F8, kind="ExternalInput").ap(),
        "sa_wv": nc.dram_tensor("sa_wv", [db, db], F8, kind="ExternalInput").ap(),
        "sa_wo": nc.dram_tensor("sa_wo", [db, db], F8, kind="ExternalInput").ap(),
        "ca_wq": nc.dram_tensor("ca_wq", [da, db], F8, kind="ExternalInput").ap(),
        "ca_wk": nc.dram_tensor("ca_wk", [db, db], F8, kind="ExternalInput").ap(),
        "ca_wv": nc.dram_tensor("ca_wv", [db, db], F8, kind="ExternalInput").ap(),
        "ca_wo": nc.dram_tensor("ca_wo", [db, db], F8, kind="ExternalInput").ap(),
        "bq_sa": nc.dram_tensor("bq_sa", [P, KB], F32, kind="ExternalInput").ap(),
        "bq_ca": nc.dram_tensor("bq_ca", [P, KB], F32, kind="ExternalInput").ap(),
    }
    outs = {"out": nc.dram_tensor("out", [n, db], F32, kind="ExternalOutput").ap()}
    with tile.TileContext(nc) as tc:
        if repeat == 1:
            build_block(tc, outs, ins, n, da, db)
        else:
            # hardware loop: same NEFF size at any repeat; an all-engine
            # barrier separates iterations (body is idempotent)
            with tc.For_i(0, repeat):
                build_block(tc, outs, ins, n, da, db)
    nc.compile()
    return nc


def prepare_maps(inputs, n=N_FULL, da=DA_FULL, db=DB_FULL):
    """Host-side prep: fp8 casts/scaling, transposes, exact bias folding.

    Returns (in_maps, add_out)."""
    f8 = ml_dtypes.float8_e4m3
    f32 = np.float32
    g = {k: np.ascontiguousarray(np.asarray(v)) for k, v in inputs.items()}
    nb = g["x_a"].shape[0]

    # exact folds (see module docstring); all biases are added in fp32
    b_eff_sa = (g["sa_bv"].astype(f32) @ g["sa_wo"].astype(f32) + g["sa_bo"].astype(f32))
    b_eff_ca = (g["ca_bv"].astype(f32) @ g["ca_wo"].astype(f32) + g["ca_bo"].astype(f32))
    xbpb = (g["x_b"].astype(f32) + b_eff_sa[None, None, :]).astype(f32)

    KB = db // P
    ws = f32(WS)
    common = {
        name: np.ascontiguousarray((ws * g[name].astype(f32)).astype(f8))
        for name in ("sa_wq", "sa_wk", "sa_wv", "sa_wo",
                     "ca_wq", "ca_wk", "ca_wv", "ca_wo")
    }
    common["bq_sa"] = np.ascontiguousarray((ws * g["sa_bq"].astype(f32)).reshape(KB, P).T)
    common["bq_ca"] = np.ascontiguousarray((ws * g["ca_bq"].astype(f32)).reshape(KB, P).T)
    in_maps = []
    for b in range(nb):
        in_maps.append(dict(
            xbT=np.ascontiguousarray(g["x_b"][b].astype(f32).T.astype(f8)),
            xaT=np.ascontiguousarray(g["x_a"][b].astype(f32).T.astype(f8)),
            xbpb=np.ascontiguousarray(xbpb[b]),
            **common,
        ))
    return in_maps, b_eff_ca


_CACHE = {}


def run_on_device(inputs, trace=False, **run_kwargs):
    """Run the full problem on 8 NeuronCores.  Returns (out [B,N,DB] f32, results)."""
    if not trace:
        # NTFF tracing needs antenv.axon_hooks, absent in this container; make
        # sure an inherited BASS_TRACE=1 can't route us into that path.
        os.environ.setdefault("BASS_NEVER_TRACE", "1")
    if "nc" not in _CACHE:
        _CACHE["nc"] = build_program()
    nc = _CACHE["nc"]
    in_maps, add_out = prepare_maps(inputs)
    res = run_bass_kernel_spmd(
        nc, in_maps, core_ids=list(range(len(in_maps))), trace=trace, **run_kwargs,
    )
    out = np.stack([r["out"] for r in res.results], axis=0)
    out = (out + add_out[None, None, :]).astype(np.float32)
    return out, res


def kernel(**inputs) -> np.ndarray:
    out, _ = run_on_device(inputs)
    return out
